# revision 27
# baseline (speedup 1.0000x reference)
"""Trainium2 Bass kernel for nn_BlockAttentionResidual.

Reference semantics (per (b, t) position):
    inv_rms_n = rsqrt(mean_d(x_n^2) + eps)                 n = 0..7 sources
    score_n   = dot(q, x_n) * inv_rms_n / sqrt(D)          q = w_query * norm_weight
    w         = softmax_n(score_n)
    out       = sum_n w_n * x_n                            [D]

Sharding: 8192 (b,t) tokens split contiguously across 8 cores (1024 each).
Per core, tokens are processed in 8 "super-iterations" of 128 tokens; each
super-iteration is 8 SBUF tiles of [128 rows = 16 tokens x 8 sources, D=2048].

Each tile streams through a per-tile pipeline (load -> reductions -> scores
-> PE matmuls) so its SBUF slot frees as soon as its own matmuls retire,
which keeps the DMA prefetch queue flowing; measured at the pure-DMA roofline
(~240 us/core for the 72 MiB of traffic, ~315 GB/s/core).

Per-row reductions over D (sum x^2 and dot(q, x)) are single-pass fused ops:
  - ScalarE activation(Square, accum_out=...)        -> sumsq
  - VectorE scalar_tensor_tensor(mult, mult, accum)  -> dot
Softmax skips max-subtraction: |score| <= |q| ~ 0.9 (Cauchy-Schwarz), so exp
is safe.  1/sqrt is computed as exp(-0.5*ln(v)) to stay in one ACT table set.
The weighted combine runs on the PE as 8 PSUM-accumulated matmuls W_j.T @ X_j
with W_j a [128, 128] block-diagonal scatter of exp(score) (built by one
tensor_scalar_mul against a constant mask), in float32r (full-rate fp32
matmul).  The softmax denominator Z accumulates from W_j.T @ ones, and the
PSUM->SBUF eviction applies the 1/Z normalization via a per-partition
activation scale; the store issues from the scalar-engine HWDGE queue so its
wait never stalls the sync queue's load triggers.
"""

import numpy as np

import concourse.bass as bass
import concourse.tile as tile
from concourse import mybir
from concourse.bass_utils import run_bass_kernel_spmd

# Extra kwargs for run_bass_kernel_spmd (test harness sets {"trace": True});
# the last BassKernelResults is stashed for timing inspection.
_run_kwargs = {}
_last_results = None

B, T, N, D = 2, 4096, 8, 2048
EPS = 1e-6
NCORES = 8
TOK = (B * T) // NCORES          # tokens per core = 1024
SUPER = 128                      # tokens per super-iteration
G = TOK // SUPER                 # super-iterations per core = 8
TPT = 128 // N                   # tokens per tile = 16
J = SUPER // TPT                 # tiles per super-iteration = 8

F32 = mybir.dt.float32
F32R = mybir.dt.float32r
FT = mybir.ActivationFunctionType
OP = mybir.AluOpType



def _split_multi_waits(nc: bass.Bass, limit: int = 1) -> None:
    """Move surplus sync waits onto same-engine NoOp carriers.

    This walrus build accepts only one sync-wait slot per ISA instruction;
    Tile can attach several.  A NoOp on the same engine executed immediately
    before the instruction enforces the same AND-of-waits semantics.
    """
    k = 0
    for func in nc.m.functions:
        for blk in func.blocks:
            new_insts = []
            for inst in blk.instructions:
                si = inst.sync_info
                ow = list(si.on_wait) if si is not None and si.on_wait else []
                if len(ow) > limit:
                    for w in ow[:-limit]:
                        nop = mybir.InstNoOp(
                            name=f"waitnop-{k}",
                            sync_info=mybir.SyncInfo(on_wait=[w], on_update=[]),
                            bass_nofuse=True,
                            engine=inst.engine,
                        )
                        k += 1
                        new_insts.append(nop)
                    si.on_wait = ow[-limit:]
                new_insts.append(inst)
            if len(new_insts) != len(blk.instructions):
                blk.instructions[:] = new_insts


def build_nc(split_waits: bool = True, loop_n: int | None = None) -> bass.Bass:
    nc = bass.Bass()
    src = nc.declare_dram_parameter("src", [TOK * N, D], F32, isOutput=False)
    qv = nc.declare_dram_parameter("qv", [D], F32, isOutput=False)
    maskp = nc.declare_dram_parameter("maskp", [128, J * 128], F32, isOutput=False)
    onesp = nc.declare_dram_parameter("onesp", [128, 2], F32, isOutput=False)
    out = nc.declare_dram_parameter("out", [TOK, D], F32, isOutput=True)

    src_t = src.rearrange("(g j p) d -> g j p d", g=G, j=J, p=128)
    out_t = out.rearrange("(g p) d -> g p d", p=128)

    with tile.TileContext(nc) as tc:
        with (
            tc.tile_pool(name="singles", bufs=1) as singles,
            tc.tile_pool(name="xpool", bufs=16) as xpool,
            tc.tile_pool(name="scratch_a", bufs=1) as scr_a,
            tc.tile_pool(name="scratch_v", bufs=1) as scr_v,
            tc.tile_pool(name="spool", bufs=2) as spool,
            tc.tile_pool(name="wpool", bufs=4) as wpool,
            tc.tile_pool(name="opool", bufs=2) as opool,
            tc.tile_pool(name="psum_o", bufs=1, space="PSUM") as psum_o_pool,
            tc.tile_pool(name="psum_z", bufs=2, space="PSUM") as psum_z_pool,
        ):
            # ---- one-time constants ----
            qb = singles.tile([128, D], F32)
            nc.sync.dma_start(out=qb, in_=qv[None, :].to_broadcast([128, D]))

            mask = singles.tile([128, J * 128], F32)
            nc.sync.dma_start(out=mask, in_=maskp[:, :])

            ones_col = singles.tile([128, 2], F32R)
            nc.sync.dma_start(out=ones_col, in_=onesp[:, :].bitcast(F32R))

            bias_eps = singles.tile([128, 1], F32)
            nc.vector.memset(bias_eps, EPS * D)
            bias_zero = singles.tile([128, 1], F32)
            nc.vector.memset(bias_zero, 0.0)

            # Touch qb on VectorE once so later DVE consumers inherit the
            # dependency via engine program order instead of extra sem waits
            # (the TensorScalarPtr ISA slot has a tight wait budget).
            probe = singles.tile([128, 1], F32)
            nc.vector.tensor_copy(probe, qb[:, 0:1])

            import contextlib

            loop_cm = (
                tc.For_i(0, loop_n, 1) if loop_n is not None
                else contextlib.nullcontext()
            )
            with loop_cm:
              for g in range(G):
                # Per-tile streaming: each tile is loaded, reduced, scored,
                # and fed to the PE immediately, so its SBUF slot frees as
                # soon as its own matmuls retire (keeps DMA prefetch flowing).
                po = psum_o_pool.tile([128, D], F32)
                pz = psum_z_pool.tile([128, 2], F32)
                for j in range(J):
                    xt = xpool.tile([128, D], F32R)
                    nc.sync.dma_start(out=xt, in_=src_t[g, j].bitcast(F32R))

                    sums = spool.tile([128, 1], F32, tag="sums")
                    dots = spool.tile([128, 1], F32, tag="dots")
                    sq_scr = scr_a.tile([128, D], F32, tag="sq")
                    nc.scalar.activation(
                        out=sq_scr,
                        in_=xt.bitcast(F32),
                        func=FT.Square,
                        accum_out=sums,
                    )
                    tt_scr = scr_v.tile([128, D], F32, tag="tt")
                    nc.vector.scalar_tensor_tensor(
                        out=tt_scr,
                        in0=xt.bitcast(F32),
                        scalar=1.0,
                        in1=qb,
                        op0=OP.mult,
                        op1=OP.mult,
                        accum_out=dots,
                    )

                    # score = dot / sqrt(sumsq + eps*D); 1/sqrt = exp(-0.5*ln)
                    lnv = spool.tile([128, 1], F32, tag="lnv")
                    nc.scalar.activation(
                        out=lnv, in_=sums, func=FT.Ln, bias=bias_eps, scale=1.0
                    )
                    rhat = spool.tile([128, 1], F32, tag="rhat")
                    nc.scalar.activation(
                        out=rhat, in_=lnv, func=FT.Exp, bias=bias_zero, scale=-0.5
                    )
                    scores = spool.tile([128, 1], F32, tag="scores")
                    nc.vector.tensor_mul(scores, dots, rhat)
                    evals = spool.tile([128, 1], F32, tag="evals")
                    nc.scalar.activation(
                        out=evals, in_=scores, func=FT.Exp, bias=bias_zero
                    )

                    w = wpool.tile([128, 128], F32R, tag="w")
                    nc.vector.tensor_scalar_mul(
                        w, mask[:, 128 * j : 128 * (j + 1)], evals
                    )
                    for c in range(D // 512):
                        nc.tensor.matmul(
                            po[:, 512 * c : 512 * (c + 1)],
                            w,
                            xt[:, 512 * c : 512 * (c + 1)],
                            start=(j == 0),
                            stop=(j == J - 1),
                        )
                    nc.tensor.matmul(
                        pz, w, ones_col, start=(j == 0), stop=(j == J - 1)
                    )

                # ---- normalize by Z during PSUM eviction, then store ----
                invz = spool.tile([128, 1], F32, tag="invz")
                nc.vector.reciprocal(invz, pz[:, 0:1])
                ot = opool.tile([128, D], F32)
                nc.scalar.activation(out=ot, in_=po, func=FT.Copy, scale=invz)
                # Store via the scalar-engine HWDGE queue: its wait (evict
                # done) is satisfied by ACT program order, so it never blocks
                # the sync queue's load triggers for the next super-iter.
                nc.scalar.dma_start(out=out_t[g], in_=ot)

    if split_waits:
        _split_multi_waits(nc)
    return nc


def make_mask() -> np.ndarray:
    """Block-diagonal weight scatter masks, one [128, 128] block per tile j.

    Block j has mask[p, TPT*j + p // N] = 1: row p of tile j (= token p//N,
    source p%N) contributes to output token TPT*j + p//N of the super-iter.
    """
    m = np.zeros((128, J * 128), dtype=np.float32)
    for j in range(J):
        for p in range(128):
            m[p, 128 * j + TPT * j + p // N] = 1.0
    return m


def kernel(sources, w_query, norm_weight):
    sources = np.asarray(sources, dtype=np.float32)
    w_query = np.asarray(w_query, dtype=np.float32)
    norm_weight = np.asarray(norm_weight, dtype=np.float32)

    nc = build_nc()

    q = np.ascontiguousarray(w_query * norm_weight)
    flat = np.ascontiguousarray(sources.reshape(B * T * N, D))
    mask_np = make_mask()
    ones_np = np.ones((128, 2), dtype=np.float32)
    in_maps = [
        {"src": flat[c * TOK * N : (c + 1) * TOK * N], "qv": q, "maskp": mask_np,
         "onesp": ones_np}
        for c in range(NCORES)
    ]
    global _last_results
    res = run_bass_kernel_spmd(nc, in_maps, list(range(NCORES)), **_run_kwargs)
    _last_results = res
    outs = [res.results[c]["out"] for c in range(NCORES)]
    return np.concatenate(outs, axis=0).reshape(B, T, D).astype(np.float32)



# revision 32
# speedup vs baseline: 1.1922x; 1.1922x over previous
"""Trainium2 Bass kernel for nn_BlockAttentionResidual.

Reference semantics (per (b, t) position):
    inv_rms_n = rsqrt(mean_d(x_n^2) + eps)                 n = 0..7 sources
    score_n   = dot(q, x_n) * inv_rms_n / sqrt(D)          q = w_query * norm_weight
    w         = softmax_n(score_n)
    out       = sum_n w_n * x_n                            [D]

Sharding: 8192 (b,t) tokens split contiguously across 8 cores (1024 each).
Per core, tokens are processed in 8 "super-iterations" of 128 tokens; each
super-iteration is 8 SBUF tiles of [128 rows = 16 tokens x 8 sources, D=2048].

Tiles stream through a pipeline (load -> fused reductions -> scores -> PE
matmuls) in score-batches of batch_q=2 tiles, so each tile's SBUF slot frees
shortly after its own matmuls retire (keeps the in-order sync-queue DMA
prefetch flowing) while the tiny [128, Q] score ops amortize ScalarE's
per-instruction overhead.  Measured at the pure-DMA roofline for the 72 MiB
of per-core traffic (~220-240 us/core depending on terminal load, ~330 GB/s).

Per-row reductions over D (sum x^2 and dot(q, x)) are single-pass fused ops:
  - ScalarE activation(Square, accum_out=...)        -> sumsq
  - VectorE scalar_tensor_tensor(mult, mult, accum)  -> dot
Softmax skips max-subtraction: |score| <= |q| ~ 0.9 (Cauchy-Schwarz), so exp
is safe.  1/sqrt is computed as exp(-0.5*ln(v)) to stay in one ACT table set.
The weighted combine runs on the PE as 8 PSUM-accumulated matmuls W_j.T @ X_j
with W_j a [128, 128] block-diagonal scatter of exp(score) (built by one
tensor_scalar_mul against a constant mask), in float32r (full-rate fp32
matmul).  The softmax denominator Z accumulates from W_j.T @ ones, and the
PSUM->SBUF eviction applies the 1/Z normalization via a per-partition
activation scale; the store issues from the scalar-engine HWDGE queue so its
wait never stalls the sync queue's load triggers.
"""

import numpy as np

import concourse.bass as bass
import concourse.tile as tile
from concourse import mybir
from concourse.bass_utils import run_bass_kernel_spmd

# Extra kwargs for run_bass_kernel_spmd (test harness sets {"trace": True});
# the last BassKernelResults is stashed for timing inspection.
_run_kwargs = {}
_last_results = None

B, T, N, D = 2, 4096, 8, 2048
EPS = 1e-6
NCORES = 8
TOK = (B * T) // NCORES          # tokens per core = 1024
SUPER = 128                      # tokens per super-iteration
G = TOK // SUPER                 # super-iterations per core = 8
TPT = 128 // N                   # tokens per tile = 16
J = SUPER // TPT                 # tiles per super-iteration = 8

F32 = mybir.dt.float32
F32R = mybir.dt.float32r
FT = mybir.ActivationFunctionType
OP = mybir.AluOpType



def _split_multi_waits(nc: bass.Bass, limit: int = 1) -> None:
    """Move surplus sync waits onto same-engine NoOp carriers.

    This walrus build accepts only one sync-wait slot per ISA instruction;
    Tile can attach several.  A NoOp on the same engine executed immediately
    before the instruction enforces the same AND-of-waits semantics.
    """
    k = 0
    for func in nc.m.functions:
        for blk in func.blocks:
            new_insts = []
            for inst in blk.instructions:
                si = inst.sync_info
                ow = list(si.on_wait) if si is not None and si.on_wait else []
                if len(ow) > limit:
                    for w in ow[:-limit]:
                        nop = mybir.InstNoOp(
                            name=f"waitnop-{k}",
                            sync_info=mybir.SyncInfo(on_wait=[w], on_update=[]),
                            bass_nofuse=True,
                            engine=inst.engine,
                        )
                        k += 1
                        new_insts.append(nop)
                    si.on_wait = ow[-limit:]
                new_insts.append(inst)
            if len(new_insts) != len(blk.instructions):
                blk.instructions[:] = new_insts


def build_nc(split_waits: bool = True, loop_n: int | None = None, batch_q: int = 2) -> bass.Bass:
    nc = bass.Bass()
    src = nc.declare_dram_parameter("src", [TOK * N, D], F32, isOutput=False)
    qv = nc.declare_dram_parameter("qv", [D], F32, isOutput=False)
    maskp = nc.declare_dram_parameter("maskp", [128, J * 128], F32, isOutput=False)
    onesp = nc.declare_dram_parameter("onesp", [128, 2], F32, isOutput=False)
    out = nc.declare_dram_parameter("out", [TOK, D], F32, isOutput=True)

    src_t = src.rearrange("(g j p) d -> g j p d", g=G, j=J, p=128)
    out_t = out.rearrange("(g p) d -> g p d", p=128)

    with tile.TileContext(nc) as tc:
        with (
            tc.tile_pool(name="singles", bufs=1) as singles,
            tc.tile_pool(name="xpool", bufs=18) as xpool,
            tc.tile_pool(name="scratch_a", bufs=1) as scr_a,
            tc.tile_pool(name="scratch_v", bufs=1) as scr_v,
            tc.tile_pool(name="spool", bufs=2) as spool,
            tc.tile_pool(name="wpool", bufs=4) as wpool,
            tc.tile_pool(name="opool", bufs=2) as opool,
            tc.tile_pool(name="psum_o", bufs=1, space="PSUM") as psum_o_pool,
            tc.tile_pool(name="psum_z", bufs=2, space="PSUM") as psum_z_pool,
        ):
            # ---- one-time constants ----
            qb = singles.tile([128, D], F32)
            nc.sync.dma_start(out=qb, in_=qv[None, :].to_broadcast([128, D]))

            mask = singles.tile([128, J * 128], F32)
            nc.sync.dma_start(out=mask, in_=maskp[:, :])

            ones_col = singles.tile([128, 2], F32R)
            nc.sync.dma_start(out=ones_col, in_=onesp[:, :].bitcast(F32R))

            bias_eps = singles.tile([128, 1], F32)
            nc.vector.memset(bias_eps, EPS * D)
            bias_zero = singles.tile([128, 1], F32)
            nc.vector.memset(bias_zero, 0.0)

            # Touch qb on VectorE once so later DVE consumers inherit the
            # dependency via engine program order instead of extra sem waits
            # (the TensorScalarPtr ISA slot has a tight wait budget).
            probe = singles.tile([128, 1], F32)
            nc.vector.tensor_copy(probe, qb[:, 0:1])

            import contextlib

            loop_cm = (
                tc.For_i(0, loop_n, 1,
                         hint_engines=(mybir.EngineType.PE,
                                       mybir.EngineType.Activation,
                                       mybir.EngineType.DVE))
                if loop_n is not None
                else contextlib.nullcontext()
            )
            with loop_cm:
              for g in range(G):
                # Per-tile streaming: each tile is loaded, reduced, scored,
                # and fed to the PE immediately, so its SBUF slot frees as
                # soon as its own matmuls retire (keeps DMA prefetch flowing).
                po = psum_o_pool.tile([128, D], F32)
                pz = psum_z_pool.tile([128, 2], F32)
                Q = batch_q  # tiles per score-batch group
                for q0 in range(0, J, Q):
                    xts = []
                    sums = spool.tile([128, Q], F32, tag="sums")
                    dots = spool.tile([128, Q], F32, tag="dots")
                    for k in range(Q):
                        j = q0 + k
                        xt = xpool.tile([128, D], F32R)
                        nc.sync.dma_start(out=xt, in_=src_t[g, j].bitcast(F32R))
                        xts.append(xt)
                        sq_scr = scr_a.tile([128, D], F32, tag="sq")
                        nc.scalar.activation(
                            out=sq_scr,
                            in_=xt.bitcast(F32),
                            func=FT.Square,
                            accum_out=sums[:, k : k + 1],
                        )
                        tt_scr = scr_v.tile([128, D], F32, tag="tt")
                        nc.vector.scalar_tensor_tensor(
                            out=tt_scr,
                            in0=xt.bitcast(F32),
                            scalar=1.0,
                            in1=qb,
                            op0=OP.mult,
                            op1=OP.mult,
                            accum_out=dots[:, k : k + 1],
                        )

                    # score = dot / sqrt(sumsq + eps*D); 1/sqrt = exp(-0.5*ln)
                    lnv = spool.tile([128, Q], F32, tag="lnv")
                    nc.scalar.activation(
                        out=lnv, in_=sums, func=FT.Ln, bias=bias_eps, scale=1.0
                    )
                    rhat = spool.tile([128, Q], F32, tag="rhat")
                    nc.scalar.activation(
                        out=rhat, in_=lnv, func=FT.Exp, bias=bias_zero, scale=-0.5
                    )
                    scores = spool.tile([128, Q], F32, tag="scores")
                    nc.vector.tensor_mul(scores, dots, rhat)
                    evals = spool.tile([128, Q], F32, tag="evals")
                    nc.scalar.activation(
                        out=evals, in_=scores, func=FT.Exp, bias=bias_zero
                    )

                    for k in range(Q):
                        j = q0 + k
                        w = wpool.tile([128, 128], F32R, tag="w")
                        nc.vector.tensor_scalar_mul(
                            w, mask[:, 128 * j : 128 * (j + 1)],
                            evals[:, k : k + 1],
                        )
                        for c in range(D // 512):
                            nc.tensor.matmul(
                                po[:, 512 * c : 512 * (c + 1)],
                                w,
                                xts[k][:, 512 * c : 512 * (c + 1)],
                                start=(j == 0),
                                stop=(j == J - 1),
                            )
                        nc.tensor.matmul(
                            pz, w, ones_col, start=(j == 0), stop=(j == J - 1)
                        )

                # ---- normalize by Z during PSUM eviction, then store ----
                invz = spool.tile([128, 1], F32, tag="invz")
                nc.vector.reciprocal(invz, pz[:, 0:1])
                ot = opool.tile([128, D], F32)
                nc.scalar.activation(out=ot, in_=po, func=FT.Copy, scale=invz)
                # Store via the scalar-engine HWDGE queue: its wait (evict
                # done) is satisfied by ACT program order, so it never blocks
                # the sync queue's load triggers for the next super-iter.
                nc.scalar.dma_start(out=out_t[g], in_=ot)

    if split_waits:
        _split_multi_waits(nc)
    return nc


def make_mask() -> np.ndarray:
    """Block-diagonal weight scatter masks, one [128, 128] block per tile j.

    Block j has mask[p, TPT*j + p // N] = 1: row p of tile j (= token p//N,
    source p%N) contributes to output token TPT*j + p//N of the super-iter.
    """
    m = np.zeros((128, J * 128), dtype=np.float32)
    for j in range(J):
        for p in range(128):
            m[p, 128 * j + TPT * j + p // N] = 1.0
    return m


def kernel(sources, w_query, norm_weight):
    sources = np.asarray(sources, dtype=np.float32)
    w_query = np.asarray(w_query, dtype=np.float32)
    norm_weight = np.asarray(norm_weight, dtype=np.float32)

    nc = build_nc()

    q = np.ascontiguousarray(w_query * norm_weight)
    flat = np.ascontiguousarray(sources.reshape(B * T * N, D))
    mask_np = make_mask()
    ones_np = np.ones((128, 2), dtype=np.float32)
    in_maps = [
        {"src": flat[c * TOK * N : (c + 1) * TOK * N], "qv": q, "maskp": mask_np,
         "onesp": ones_np}
        for c in range(NCORES)
    ]
    global _last_results
    res = run_bass_kernel_spmd(nc, in_maps, list(range(NCORES)), **_run_kwargs)
    _last_results = res
    outs = [res.results[c]["out"] for c in range(NCORES)]
    return np.concatenate(outs, axis=0).reshape(B, T, D).astype(np.float32)

